# revision 1
# baseline (speedup 1.0000x reference)
"""Per-pixel dynamic 5x5 conv (kernel-estimation) for TRN2, 8 NeuronCores.

Semantics (matches the reference):
  out[n,c,h,w] = leaky_relu( sum_{i,j} K6[n,c,h,w,i,j] * xpad[n,c,h+i,w+j], 0.2 )
where K6 = kernel.reshape(N, C, H, W, 5, 5) (row-major) and xpad is the
replication-padded input (pad=2 each side).

Sharding: the 32 (n,c) pairs are fully independent -> 4 pairs per core.
Host pre-pads x (edge) and reshapes kernel to (pairs, H, W, 25) views.

Per-core bass kernel layout:
  - h in partitions (2 chunks of 128 rows), (pair, w) in the free dim.
  - coef tile [128, 4, WB*25] DMA'd contiguously; tap t is a stride-25 view.
  - x: 5 row-shifted tiles [128, 4, 260] (tap (i,j) -> tile i, free offset j).
  - per tap: DVE mul -> prod; most taps accumulate via PE identity-matmul
    (fp32, exact) into PSUM; the rest via DVE adds into an SBUF acc that is
    merged into PSUM by one final matmul.
  - ACT applies LeakyRelu(0.2) PSUM->SBUF, then DMA out.
"""

import sys

import numpy as np

sys.path.insert(0, "/opt/trn_rl_repo")

N, C, H, W = 4, 8, 256, 256
KS = 5
PAD = (KS - 1) // 2  # 2
TAPS = KS * KS  # 25
NCORES = 8
PAIRS = (N * C) // NCORES  # 4 (n,c) pairs per core
HP, WP = H + 2 * PAD, W + 2 * PAD  # 260, 260
PPART = 128  # partitions
NCHUNK = H // PPART  # 2 h-chunks
WB = 128  # w-block width; free dim per op = PAIRS*WB = 512
NWB = W // WB
# taps accumulated on the PE (identity matmul, fp32 = 4 cyc/row); the rest
# are accumulated with DVE adds. Balances DVE vs PE busy time.
N_PE_TAPS = 21

_CACHE = {}


def _split_multi_waits(nc, mybir):
    """TRN2 compute/DMA instructions encode at most one sync-wait command;
    Tile can attach several. Hoist extras into standalone EventSemaphore
    waits (same engine, immediately before) — identical blocking semantics.
    """
    for fn in nc.m.functions:
        for blk in fn.blocks:
            insts = blk.instructions
            out = []
            for inst in insts:
                si = inst.sync_info
                if (
                    si is not None
                    and len(si.on_wait) > 1
                    and not isinstance(inst, mybir.InstEventSemaphore)
                ):
                    waits = list(si.on_wait)
                    for w in waits[:-1]:
                        out.append(
                            mybir.InstEventSemaphore(
                                name=nc.get_next_instruction_name(),
                                engine=inst.engine,
                                sync_info=mybir.SyncInfo(
                                    on_wait=[w], on_update=[]
                                ),
                            )
                        )
                    inst.sync_info = mybir.SyncInfo(
                        on_wait=[waits[-1]], on_update=list(si.on_update)
                    )
                out.append(inst)
            insts[:] = out


def _build():
    import concourse.bass as bass
    import concourse.mybir as mybir
    from concourse.bass_types import AP
    from concourse.tile import TileContext

    f32 = mybir.dt.float32
    nc = bass.Bass(trn_type="TRN2")

    xp = nc.dram_tensor("xp", (PAIRS, HP, WP), f32, kind="ExternalInput")
    kc = nc.dram_tensor("kc", (PAIRS, H, W, TAPS), f32, kind="ExternalInput")
    ident = nc.dram_tensor("ident", (PPART, PPART), f32, kind="ExternalInput")
    out = nc.dram_tensor("out", (PAIRS, H, W), f32, kind="ExternalOutput")

    xp_h = xp[:].rearrange("a h w -> h a w")  # [260, 4, 260]
    kc_h = kc[:].rearrange("a h w t -> h a w t")  # [256, 4, 256, 25]
    out_h = out[:].rearrange("a h w -> h a w")  # [256, 4, 256]

    pe_taps = list(range(N_PE_TAPS))
    dve_taps = list(range(N_PE_TAPS, TAPS))

    with TileContext(nc) as tc:
        with (
            tc.tile_pool(name="const", bufs=1) as cpool,
            tc.tile_pool(name="xtiles", bufs=2) as xpool,
            tc.tile_pool(name="coef", bufs=2) as kpool,
            tc.tile_pool(name="prod", bufs=8) as ppool,
            tc.tile_pool(name="acc", bufs=2) as apool,
            tc.tile_pool(name="outs", bufs=2) as opool,
            tc.tile_pool(name="anchor", bufs=1) as npool,
            tc.tile_pool(name="ps", bufs=2, space="PSUM") as pspool,
        ):
            id_t = cpool.tile([PPART, PPART], f32)
            nc.sync.dma_start(id_t[:], ident[:])

            for ch in range(NCHUNK):
                h0 = ch * PPART
                # one DMA for the whole 5-row sliding window: for each
                # (partition p, pair a) the rows h0+p .. h0+p+4 are one
                # contiguous KS*WP-element run in DRAM.
                # xt[p, a, i*WP + w] = xp[a, h0 + p + i, w]
                xt = xpool.tile([PPART, PAIRS, KS * WP], f32, tag="x")
                base = xp_h[h0 : h0 + PPART]  # offset in canonical units
                x_src = AP(
                    base.tensor,
                    base.offset,
                    [[WP, PPART], [HP * WP, PAIRS], [1, KS * WP]],
                )
                nc.sync.dma_start(xt[:], x_src)
                for wb in range(NWB):
                    w0 = wb * WB
                    coef = kpool.tile([PPART, PAIRS, WB * TAPS], f32)
                    nc.sync.dma_start(
                        coef[:].rearrange("p a (w t) -> p a w t", t=TAPS),
                        kc_h[h0 : h0 + PPART, :, w0 : w0 + WB, :],
                    )
                    coef4 = coef[:].rearrange("p a (w t) -> p a w t", t=TAPS)
                    psum = pspool.tile([PPART, PAIRS * WB], f32)
                    acc = apool.tile([PPART, PAIRS, WB], f32)

                    # anchor: absorbs the coef-DMA + x-DMA waits in one cheap
                    # DVE op so later instructions carry <=2 sync waits.
                    anch = npool.tile([1, 2], f32, tag="anchor")
                    nc.vector.tensor_tensor(
                        anch[:],
                        coef[0:1, 0:1, 0:2].rearrange("p a w -> p (a w)"),
                        xt[0:1, 0:1, 0:2].rearrange("p a w -> p (a w)"),
                        mybir.AluOpType.add,
                    )

                    first_pe = True
                    first_dve = True
                    for t in range(TAPS):
                        i, j = divmod(t, KS)
                        c_ap = coef4[:, :, :, t]
                        xoff = i * WP + w0 + j
                        x_ap = xt[:, :, xoff : xoff + WB]
                        if t in dve_taps and first_dve:
                            nc.vector.tensor_mul(acc[:], c_ap, x_ap)
                            first_dve = False
                            continue
                        prod = ppool.tile([PPART, PAIRS, WB], f32)
                        nc.vector.tensor_mul(prod[:], c_ap, x_ap)
                        prod2 = prod[:].rearrange("p a w -> p (a w)")
                        if t in pe_taps:
                            nc.tensor.matmul(
                                psum[:], id_t[:], prod2,
                                start=first_pe, stop=False,
                            )
                            first_pe = False
                        else:
                            nc.vector.tensor_add(acc[:], acc[:], prod[:])
                    # merge the DVE accumulator into PSUM (last matmul in group)
                    nc.tensor.matmul(
                        psum[:], id_t[:],
                        acc[:].rearrange("p a w -> p (a w)"),
                        start=first_pe, stop=True,
                    )
                    # leaky_relu(x, 0.2) = max(0.2*x, x); the HW Lrelu table
                    # has a baked-in 0.01 slope, so compute it explicitly.
                    o_s = opool.tile([PPART, PAIRS * WB], f32, tag="oscale")
                    nc.scalar.activation(
                        o_s[:], psum[:],
                        mybir.ActivationFunctionType.Copy, scale=0.2,
                    )
                    o_t = opool.tile([PPART, PAIRS, WB], f32, tag="out")
                    nc.vector.tensor_max(
                        o_t[:].rearrange("p a w -> p (a w)"), o_s[:], psum[:]
                    )
                    nc.sync.dma_start(
                        out_h[h0 : h0 + PPART, :, w0 : w0 + WB], o_t[:]
                    )
    _split_multi_waits(nc, mybir)
    return nc


def _get_nc():
    if "nc" not in _CACHE:
        _CACHE["nc"] = _build()
    return _CACHE["nc"]


def kernel(input, kernel):
    x = np.asarray(input, dtype=np.float32)
    kern = np.asarray(kernel, dtype=np.float32)

    xpad = np.pad(x, ((0, 0), (0, 0), (PAD, PAD), (PAD, PAD)), mode="edge")
    k6 = kern.reshape(N, C, H, W, TAPS)
    ident = np.eye(PPART, dtype=np.float32)

    in_maps = []
    for core in range(NCORES):
        n = core // 2
        c0 = (core % 2) * PAIRS
        in_maps.append(
            {
                "xp": np.ascontiguousarray(xpad[n, c0 : c0 + PAIRS]),
                "kc": np.ascontiguousarray(k6[n, c0 : c0 + PAIRS]),
                "ident": ident,
            }
        )

    from concourse.bass_utils import run_bass_kernel_spmd

    res = run_bass_kernel_spmd(_get_nc(), in_maps, core_ids=list(range(NCORES)))

    out = np.empty((N, C, H, W), dtype=np.float32)
    for core in range(NCORES):
        n = core // 2
        c0 = (core % 2) * PAIRS
        out[n, c0 : c0 + PAIRS] = res.results[core]["out"]
    return out



# revision 2
# speedup vs baseline: 2.7894x; 2.7894x over previous
"""Per-pixel dynamic 5x5 conv for TRN2, 8 cores. v5: fine-grained streaming.

Layout: partitions = (pair, h-block) = 4 x 32 blocks of R=8 rows; free dim =
8 rows x 256 cols = 2048. x loaded once (12-row windows, bf16, split in two
DMAs so i<=2 taps start early). Coefficients (bf16, tap-major per partition)
streamed in 2-tap chunks for tight pipeline + short drain.

Per tap the product coef*x is computed in bf16 (DVE 2x mode; every ~5th tap
on GpSimd) and accumulated in fp32 PSUM via bf16 identity matmuls (4 chains
of 512 cols). A few taps accumulate on DVE into an SBUF bf16 accumulator
(merged into PSUM mid-stream) to keep PE under the DMA rate. LeakyRelu via
ACT Prelu(alpha=0.2) (exact), output bf16 in two DMAs.
"""

import sys

import numpy as np

sys.path.insert(0, "/opt/trn_rl_repo")

N, C, H, W = 4, 8, 256, 256
KS = 5
PAD = (KS - 1) // 2  # 2
TAPS = KS * KS  # 25
NCORES = 8
PAIRS = (N * C) // NCORES  # 4
WP = W + 2 * PAD  # 260
R = 8  # output rows per partition
HB = H // R  # 32
PART = PAIRS * HB  # 128
FREE = R * W  # 2048
XROWS = R + 2 * PAD  # 12
XSPLIT = 8  # first x DMA covers rows 0..7 (all i==0 taps)
MM = 512  # psum chain width (1 bank fp32)
NMM = FREE // MM  # 4

GROUPS = [1] * TAPS  # single-tap coef chunks: DVE stays arrival-paced
POOL_TAPS = {3, 7, 11, 15, 19}
ACC_TAPS = [5, 10, 14, 18]  # DVE-accumulated; merged after tap 18

_CACHE = {}


def _split_multi_waits(nc, mybir):
    """TRN2 compute/DMA instructions encode at most one sync-wait command;
    Tile can attach several. Hoist extras into standalone EventSemaphore
    waits (same engine, immediately before) - identical blocking semantics.
    """
    for fn in nc.m.functions:
        for blk in fn.blocks:
            insts = blk.instructions
            out = []
            for inst in insts:
                si = inst.sync_info
                if (
                    si is not None
                    and len(si.on_wait) > 1
                    and not isinstance(inst, mybir.InstEventSemaphore)
                ):
                    waits = list(si.on_wait)
                    for w in waits[:-1]:
                        out.append(
                            mybir.InstEventSemaphore(
                                name=nc.get_next_instruction_name(),
                                engine=inst.engine,
                                sync_info=mybir.SyncInfo(
                                    on_wait=[w], on_update=[]
                                ),
                            )
                        )
                    inst.sync_info = mybir.SyncInfo(
                        on_wait=[waits[-1]], on_update=list(si.on_update)
                    )
                out.append(inst)
            insts[:] = out


def _build():
    import concourse.bass as bass
    import concourse.mybir as mybir
    from concourse.tile import TileContext

    f32 = mybir.dt.float32
    bf16 = mybir.dt.bfloat16
    nc = bass.Bass(trn_type="TRN2")

    xh = nc.dram_tensor("xh", (PART, XROWS * WP), bf16, kind="ExternalInput")
    kc = nc.dram_tensor("kc", (PART, TAPS * FREE), bf16, kind="ExternalInput")
    ident = nc.dram_tensor("ident", (PART, PART), bf16, kind="ExternalInput")
    out = nc.dram_tensor("out", (PART, FREE), bf16, kind="ExternalOutput")

    first_acc = ACC_TAPS[0]
    last_acc = ACC_TAPS[-1]

    with TileContext(nc) as tc:
        with (
            tc.tile_pool(name="const", bufs=1) as cpool,
            tc.tile_pool(name="coef", bufs=6) as kpool,
            tc.tile_pool(name="prod", bufs=8) as ppool,
            tc.tile_pool(name="outs", bufs=1) as opool,
            tc.tile_pool(name="ps", bufs=1, space="PSUM") as pspool,
        ):
            id_t = cpool.tile([PART, PART], bf16)
            x_t = cpool.tile([PART, XROWS * WP], bf16)
            # x rows 0..9 first (covers all i<=2 taps), then the first coef
            # chunk, so the first product is gated by as little DMA as possible
            nc.sync.dma_start(
                x_t[:, : XSPLIT * WP], xh[:, : XSPLIT * WP]
            )
            xv = x_t[:].rearrange("p (r w) -> p r w", w=WP)  # [128, 12, 260]

            psums = [
                pspool.tile([PART, MM], f32, tag=f"ps{s}", name=f"psum{s}")
                for s in range(NMM)
            ]
            acc = cpool.tile([PART, FREE], bf16, name="acc")
            acc3 = acc[:].rearrange("p (r w) -> p r w", w=W)
            o_t = opool.tile([PART, FREE], bf16)

            # pool products arrive ~4us after their chunk; defer their
            # matmuls two taps so PE's in-order stream never stalls on them
            pending_pool = []

            def flush_pool(now, force=False):
                while pending_pool and (
                    force or now - pending_pool[0][0] >= 2
                ):
                    _, pprod = pending_pool.pop(0)
                    for s in range(NMM):
                        nc.tensor.matmul(
                            psums[s][:],
                            id_t[:],
                            pprod[:, s * MM : (s + 1) * MM],
                            start=False,
                            stop=False,
                        )

            t = 0
            for g, gsz in enumerate(GROUPS):
                ct = kpool.tile([PART, gsz * FREE], bf16, tag="coef", name="ct")
                nc.sync.dma_start(
                    ct[:], kc[:, t * FREE : (t + gsz) * FREE]
                )
                if g == 0:
                    nc.sync.dma_start(id_t[:], ident[:])
                    # x rows 8-11, needed only by taps with i >= 1 (t >= 5)
                    nc.sync.dma_start(
                        x_t[:, XSPLIT * WP :], xh[:, XSPLIT * WP :]
                    )
                for tl in range(gsz):
                    i, j = divmod(t, KS)
                    c3 = ct[:, tl * FREE : (tl + 1) * FREE].rearrange(
                        "p (r w) -> p r w", w=W
                    )
                    x3 = xv[:, i : i + R, j : j + W]
                    if t == TAPS - 1:
                        flush_pool(t, force=True)
                        # final tap: per-slice mul -> stop matmul -> Prelu ->
                        # out DMA pipeline (drains the chains incrementally)
                        prod = ppool.tile(
                            [PART, FREE], bf16, tag="prod", name="prod"
                        )
                        RS = MM // W  # output rows per 512-col slice
                        for s in range(NMM):
                            nc.vector.tensor_mul(
                                prod[:, s * MM : (s + 1) * MM].rearrange(
                                    "p (r w) -> p r w", w=W
                                ),
                                ct[:, tl * FREE + s * MM
                                   : tl * FREE + (s + 1) * MM].rearrange(
                                    "p (r w) -> p r w", w=W
                                ),
                                xv[:, i + s * RS : i + s * RS + RS, j : j + W],
                            )
                            nc.tensor.matmul(
                                psums[s][:],
                                id_t[:],
                                prod[:, s * MM : (s + 1) * MM],
                                start=False,
                                stop=True,
                            )
                            nc.scalar.activation(
                                o_t[:, s * MM : (s + 1) * MM],
                                psums[s][:],
                                mybir.ActivationFunctionType.Prelu,
                                alpha=0.2,
                            )
                            nc.sync.dma_start(
                                out[:, s * MM : (s + 1) * MM],
                                o_t[:, s * MM : (s + 1) * MM],
                            )
                        t += 1
                        continue
                    if t in ACC_TAPS and t == first_acc:
                        nc.vector.tensor_mul(acc3, c3, x3)
                    elif t in ACC_TAPS:
                        prod = ppool.tile(
                            [PART, FREE], bf16, tag="prod", name="prod"
                        )
                        p3 = prod[:].rearrange("p (r w) -> p r w", w=W)
                        nc.vector.tensor_mul(p3, c3, x3)
                        nc.vector.tensor_add(acc3, acc3, p3)
                    else:
                        prod = ppool.tile(
                            [PART, FREE], bf16, tag="prod", name="prod"
                        )
                        p3 = prod[:].rearrange("p (r w) -> p r w", w=W)
                        if t in POOL_TAPS:
                            nc.gpsimd.tensor_mul(p3, c3, x3)
                            pending_pool.append((t, prod))
                        else:
                            nc.vector.tensor_mul(p3, c3, x3)
                            for s in range(NMM):
                                nc.tensor.matmul(
                                    psums[s][:],
                                    id_t[:],
                                    prod[:, s * MM : (s + 1) * MM],
                                    start=(t == 0),
                                    stop=False,
                                )
                    flush_pool(t)
                    if t == last_acc:
                        # merge the DVE accumulator into each PSUM chain now;
                        # PE absorbs these while waiting for later chunks
                        for s in range(NMM):
                            nc.tensor.matmul(
                                psums[s][:],
                                id_t[:],
                                acc[:, s * MM : (s + 1) * MM],
                                start=False,
                                stop=False,
                            )
                    t += 1

    _split_multi_waits(nc, mybir)
    return nc


def _get_nc():
    if "nc" not in _CACHE:
        _CACHE["nc"] = _build()
    return _CACHE["nc"]


def kernel(input, kernel):
    import ml_dtypes

    bf16 = ml_dtypes.bfloat16
    x = np.asarray(input, dtype=np.float32)
    kern = np.asarray(kernel, dtype=np.float32)

    xpad = np.pad(x, ((0, 0), (0, 0), (PAD, PAD), (PAD, PAD)), mode="edge")
    # coef: (N, C, H, W, 25) -> (N, C, HB, t, r, w) bf16, tap-major per block
    k6 = kern.reshape(N, C, HB, R, W, TAPS)
    kc_all = np.ascontiguousarray(k6.transpose(0, 1, 2, 5, 3, 4)).astype(bf16)
    # x windows: (N, C, HB, 12, 260) bf16 (rows 8*hb .. 8*hb+11 of padded)
    sw = np.lib.stride_tricks.sliding_window_view(xpad, XROWS, axis=2)
    xh_all = np.ascontiguousarray(
        sw[:, :, :: R, :, :].transpose(0, 1, 2, 4, 3)
    ).astype(bf16)  # (N, C, HB, XROWS, WP)
    ident = np.eye(PART, dtype=np.float32).astype(bf16)

    in_maps = []
    for core in range(NCORES):
        n = core // 2
        c0 = (core % 2) * PAIRS
        in_maps.append(
            {
                "xh": np.ascontiguousarray(
                    xh_all[n, c0 : c0 + PAIRS]
                ).reshape(PART, XROWS * WP),
                "kc": np.ascontiguousarray(
                    kc_all[n, c0 : c0 + PAIRS]
                ).reshape(PART, TAPS * FREE),
                "ident": ident,
            }
        )

    from concourse.bass_utils import run_bass_kernel_spmd

    res = run_bass_kernel_spmd(_get_nc(), in_maps, core_ids=list(range(NCORES)))

    out = np.empty((N, C, H, W), dtype=np.float32)
    for core in range(NCORES):
        n = core // 2
        c0 = (core % 2) * PAIRS
        out[n, c0 : c0 + PAIRS] = (
            res.results[core]["out"]
            .astype(np.float32)
            .reshape(PAIRS, H, W)
        )
    return out


# revision 5
# speedup vs baseline: 3.0422x; 1.0906x over previous
"""Per-pixel dynamic 5x5 conv (kernel-estimation) for TRN2, 8 NeuronCores.

Semantics (matches the reference):
  out[n,c,h,w] = leaky_relu( sum_{i,j} K6[n,c,h,w,i,j] * xpad[n,c,h+i,w+j], 0.2 )
The 32 (n,c) pairs are independent -> 4 per core.

Layout: partitions = (pair, h-block) = 4 x 32 blocks of R=8 rows; free dim =
8 rows x 256 cols = 2048. x is loaded once as 12-row windows (bf16, 1.5x
halo, split in two DMAs so i=0 taps start early). Coefficients stream in
single-tap chunks, tap-major per partition: bf16 for DVE taps, fp8e4m3 for
the six GpSimd taps (halves their DMA bytes; GpSimd has no 2x mode to lose;
measured rel err 1.34e-2 < 2e-2 gate).

Per tap the product coef*x is computed in bf16 (DVE 2x mode, 19 taps; GpSimd
6 taps with matmuls deferred two taps so PE's in-order stream never stalls)
and accumulated exactly in fp32 PSUM via bf16 identity matmuls (4 chains of
512 cols = 1 bank each). Four taps accumulate on DVE into an SBUF bf16
accumulator merged into PSUM mid-stream, keeping PE under the DMA rate.
The final tap runs per-slice (mul -> stop matmul -> ACT Prelu(alpha=0.2),
which is exact leaky_relu on HW -> out DMA) to pipeline the drain.
Output returned as bf16, upcast to f32 on host.

DMA stream is gapless at the 360 B/ns roofline: ~35.9us of traffic
(coef 11.5MB + x 0.8MB + out 0.5MB per core), ~44.8us total.
"""

import sys

import numpy as np

sys.path.insert(0, "/opt/trn_rl_repo")

N, C, H, W = 4, 8, 256, 256
KS = 5
PAD = (KS - 1) // 2  # 2
TAPS = KS * KS  # 25
NCORES = 8
PAIRS = (N * C) // NCORES  # 4
WP = W + 2 * PAD  # 260
R = 8  # output rows per partition
HB = H // R  # 32
PART = PAIRS * HB  # 128
FREE = R * W  # 2048
XROWS = R + 2 * PAD  # 12
XSPLIT = 8  # first x DMA covers rows 0..7 (all i==0 taps)
MM = 512  # psum chain width (1 bank fp32)
NMM = FREE // MM  # 4

GROUPS = [1] * TAPS  # single-tap coef chunks: DVE stays arrival-paced
# pool taps stream their coefficients in fp8e4m3 (half the DMA bytes; Pool
# has no 2x mode so fp8 costs it nothing; measured rel err ~1.2e-2 < 2e-2)
POOL_TAPS = (2, 5, 8, 11, 14, 17)
ACC_TAPS = [4, 9, 13, 18]  # DVE-accumulated; merged after tap 18

_CACHE = {}


def _split_multi_waits(nc, mybir):
    """TRN2 compute/DMA instructions encode at most one sync-wait command;
    Tile can attach several. Hoist extras into standalone EventSemaphore
    waits (same engine, immediately before) - identical blocking semantics.
    """
    for fn in nc.m.functions:
        for blk in fn.blocks:
            insts = blk.instructions
            out = []
            for inst in insts:
                si = inst.sync_info
                if (
                    si is not None
                    and len(si.on_wait) > 1
                    and not isinstance(inst, mybir.InstEventSemaphore)
                ):
                    waits = list(si.on_wait)
                    for w in waits[:-1]:
                        out.append(
                            mybir.InstEventSemaphore(
                                name=nc.get_next_instruction_name(),
                                engine=inst.engine,
                                sync_info=mybir.SyncInfo(
                                    on_wait=[w], on_update=[]
                                ),
                            )
                        )
                    inst.sync_info = mybir.SyncInfo(
                        on_wait=[waits[-1]], on_update=list(si.on_update)
                    )
                out.append(inst)
            insts[:] = out


def _build():
    import concourse.bass as bass
    import concourse.mybir as mybir
    from concourse.tile import TileContext

    f32 = mybir.dt.float32
    bf16 = mybir.dt.bfloat16
    nc = bass.Bass(trn_type="TRN2")

    fp8 = mybir.dt.float8e4
    xh = nc.dram_tensor("xh", (PART, XROWS * WP), bf16, kind="ExternalInput")
    kc = nc.dram_tensor("kc", (PART, TAPS * FREE), bf16, kind="ExternalInput")
    kc8 = nc.dram_tensor(
        "kc8", (PART, len(POOL_TAPS) * FREE), fp8, kind="ExternalInput"
    )
    ident = nc.dram_tensor("ident", (PART, PART), bf16, kind="ExternalInput")
    out = nc.dram_tensor("out", (PART, FREE), bf16, kind="ExternalOutput")

    first_acc = ACC_TAPS[0]
    last_acc = ACC_TAPS[-1]

    with TileContext(nc) as tc:
        with (
            tc.tile_pool(name="const", bufs=1) as cpool,
            tc.tile_pool(name="coef", bufs=6) as kpool,
            tc.tile_pool(name="prod", bufs=8) as ppool,
            tc.tile_pool(name="outs", bufs=1) as opool,
            tc.tile_pool(name="ps", bufs=1, space="PSUM") as pspool,
        ):
            id_t = cpool.tile([PART, PART], bf16)
            x_t = cpool.tile([PART, XROWS * WP], bf16)
            # x rows 0..9 first (covers all i<=2 taps), then the first coef
            # chunk, so the first product is gated by as little DMA as possible
            nc.sync.dma_start(
                x_t[:, : XSPLIT * WP], xh[:, : XSPLIT * WP]
            )
            xv = x_t[:].rearrange("p (r w) -> p r w", w=WP)  # [128, 12, 260]

            psums = [
                pspool.tile([PART, MM], f32, tag=f"ps{s}", name=f"psum{s}")
                for s in range(NMM)
            ]
            acc = cpool.tile([PART, FREE], bf16, name="acc")
            acc3 = acc[:].rearrange("p (r w) -> p r w", w=W)
            o_t = opool.tile([PART, FREE], bf16)

            # pool products arrive ~4us after their chunk; defer their
            # matmuls two taps so PE's in-order stream never stalls on them
            pending_pool = []

            def flush_pool(now, force=False):
                while pending_pool and (
                    force or now - pending_pool[0][0] >= 2
                ):
                    _, pprod = pending_pool.pop(0)
                    for s in range(NMM):
                        nc.tensor.matmul(
                            psums[s][:],
                            id_t[:],
                            pprod[:, s * MM : (s + 1) * MM],
                            start=False,
                            stop=False,
                        )

            t = 0
            for g, gsz in enumerate(GROUPS):
                if t in POOL_TAPS:
                    p8 = POOL_TAPS.index(t)
                    ct = kpool.tile(
                        [PART, gsz * FREE], fp8, tag="coef8", name="ct8"
                    )
                    nc.sync.dma_start(
                        ct[:], kc8[:, p8 * FREE : (p8 + gsz) * FREE]
                    )
                else:
                    ct = kpool.tile(
                        [PART, gsz * FREE], bf16, tag="coef", name="ct"
                    )
                    nc.sync.dma_start(
                        ct[:], kc[:, t * FREE : (t + gsz) * FREE]
                    )
                if g == 0:
                    nc.sync.dma_start(id_t[:], ident[:])
                    # x rows 8-11, needed only by taps with i >= 1 (t >= 5)
                    nc.sync.dma_start(
                        x_t[:, XSPLIT * WP :], xh[:, XSPLIT * WP :]
                    )
                for tl in range(gsz):
                    i, j = divmod(t, KS)
                    c3 = ct[:, tl * FREE : (tl + 1) * FREE].rearrange(
                        "p (r w) -> p r w", w=W
                    )
                    x3 = xv[:, i : i + R, j : j + W]
                    if t == TAPS - 1:
                        flush_pool(t, force=True)
                        # final tap: per-slice mul -> stop matmul -> Prelu ->
                        # out DMA pipeline (drains the chains incrementally)
                        prod = ppool.tile(
                            [PART, FREE], bf16, tag="prod", name="prod"
                        )
                        RS = MM // W  # output rows per 512-col slice
                        for s in range(NMM):
                            nc.vector.tensor_mul(
                                prod[:, s * MM : (s + 1) * MM].rearrange(
                                    "p (r w) -> p r w", w=W
                                ),
                                ct[:, tl * FREE + s * MM
                                   : tl * FREE + (s + 1) * MM].rearrange(
                                    "p (r w) -> p r w", w=W
                                ),
                                xv[:, i + s * RS : i + s * RS + RS, j : j + W],
                            )
                            nc.tensor.matmul(
                                psums[s][:],
                                id_t[:],
                                prod[:, s * MM : (s + 1) * MM],
                                start=False,
                                stop=True,
                            )
                            nc.scalar.activation(
                                o_t[:, s * MM : (s + 1) * MM],
                                psums[s][:],
                                mybir.ActivationFunctionType.Prelu,
                                alpha=0.2,
                            )
                            nc.sync.dma_start(
                                out[:, s * MM : (s + 1) * MM],
                                o_t[:, s * MM : (s + 1) * MM],
                            )
                        t += 1
                        continue
                    if t in ACC_TAPS and t == first_acc:
                        nc.vector.tensor_mul(acc3, c3, x3)
                    elif t in ACC_TAPS:
                        prod = ppool.tile(
                            [PART, FREE], bf16, tag="prod", name="prod"
                        )
                        p3 = prod[:].rearrange("p (r w) -> p r w", w=W)
                        nc.vector.tensor_mul(p3, c3, x3)
                        nc.vector.tensor_add(acc3, acc3, p3)
                    else:
                        prod = ppool.tile(
                            [PART, FREE], bf16, tag="prod", name="prod"
                        )
                        p3 = prod[:].rearrange("p (r w) -> p r w", w=W)
                        if t in POOL_TAPS:
                            nc.gpsimd.tensor_mul(p3, c3, x3)
                            pending_pool.append((t, prod))
                        else:
                            nc.vector.tensor_mul(p3, c3, x3)
                            for s in range(NMM):
                                nc.tensor.matmul(
                                    psums[s][:],
                                    id_t[:],
                                    prod[:, s * MM : (s + 1) * MM],
                                    start=(t == 0),
                                    stop=False,
                                )
                    flush_pool(t)
                    if t == last_acc:
                        # merge the DVE accumulator into each PSUM chain now;
                        # PE absorbs these while waiting for later chunks
                        for s in range(NMM):
                            nc.tensor.matmul(
                                psums[s][:],
                                id_t[:],
                                acc[:, s * MM : (s + 1) * MM],
                                start=False,
                                stop=False,
                            )
                    t += 1

    _split_multi_waits(nc, mybir)
    return nc


def _get_nc():
    if "nc" not in _CACHE:
        _CACHE["nc"] = _build()
    return _CACHE["nc"]


def kernel(input, kernel):
    import ml_dtypes

    bf16 = ml_dtypes.bfloat16
    x = np.asarray(input, dtype=np.float32)
    kern = np.asarray(kernel, dtype=np.float32)

    xpad = np.pad(x, ((0, 0), (0, 0), (PAD, PAD), (PAD, PAD)), mode="edge")
    # coef: (N, C, H, W, 25) -> (N, C, HB, t, r, w) bf16, tap-major per block
    fp8 = ml_dtypes.float8_e4m3fn
    k6 = kern.reshape(N, C, HB, R, W, TAPS)
    k6t = np.ascontiguousarray(k6.transpose(0, 1, 2, 5, 3, 4))
    kc_all = k6t.astype(bf16)
    kc8_all = np.ascontiguousarray(k6t[:, :, :, list(POOL_TAPS)]).astype(fp8)
    # x windows: (N, C, HB, 12, 260) bf16 (rows 8*hb .. 8*hb+11 of padded)
    sw = np.lib.stride_tricks.sliding_window_view(xpad, XROWS, axis=2)
    xh_all = np.ascontiguousarray(
        sw[:, :, :: R, :, :].transpose(0, 1, 2, 4, 3)
    ).astype(bf16)  # (N, C, HB, XROWS, WP)
    ident = np.eye(PART, dtype=np.float32).astype(bf16)

    in_maps = []
    for core in range(NCORES):
        n = core // 2
        c0 = (core % 2) * PAIRS
        in_maps.append(
            {
                "xh": np.ascontiguousarray(
                    xh_all[n, c0 : c0 + PAIRS]
                ).reshape(PART, XROWS * WP),
                "kc": np.ascontiguousarray(
                    kc_all[n, c0 : c0 + PAIRS]
                ).reshape(PART, TAPS * FREE),
                "kc8": np.ascontiguousarray(
                    kc8_all[n, c0 : c0 + PAIRS]
                ).reshape(PART, len(POOL_TAPS) * FREE),
                "ident": ident,
            }
        )

    from concourse.bass_utils import run_bass_kernel_spmd

    res = run_bass_kernel_spmd(_get_nc(), in_maps, core_ids=list(range(NCORES)))

    out = np.empty((N, C, H, W), dtype=np.float32)
    for core in range(NCORES):
        n = core // 2
        c0 = (core % 2) * PAIRS
        out[n, c0 : c0 + PAIRS] = (
            res.results[core]["out"]
            .astype(np.float32)
            .reshape(PAIRS, H, W)
        )
    return out
